# revision 1
# baseline (speedup 1.0000x reference)
"""DH-SNN network kernel for Trainium2 (8 NeuronCores, batch-parallel).

Math (per core, batch shard Bc=16):
  Wm = (W1*mask) reordered branch-major (f' = j*512+h), scaled by
       sc_j = 2*(1-alpha)*(1-beta_j); bias row b1*sc appended as K-row 700
       with a matching ones-row in x.
  DI'[f,(b,t)] = xT @ Wm''  (PE matmuls, features on partitions)
  d-scan over time per (f,b) series:  d = beta_j*d + DI'   (DVE tensor_tensor_scan,
       reset pattern data0 handles per-b series boundaries; cross-superblock
       carry injected into the tau=0 column of DI')
  som''[h,(b,t)] = sum_j d  (branch sum, scaled s.t. som'' = 2*(1-alpha)*som)
  v-loop (w = 2*v):  w_t = alpha*w_{t-1} - Y_t;   Y_{t+1} = (w_t >= 1) - som''_{t+1}
       (2 scalar_tensor_tensor ops per step; spike s_t = Y_t + som''_t)
  readout: out[b,o] = sum_t c_{t,o} * (s_t @ W2.T)[b,o] + b2[o]*sum_t c_{t,o},
       c_{t,o} = (1 - gam_o^(250-t))/250.  Linear in s => P = W2T@(Y+som'')
       accumulated on PE, then weighted time-reduce on DVE.
"""

import os
import numpy as np

B_FULL, T, INP, H, NB, OUT = 128, 250, 700, 512, 4, 20
NCORES = 8
BC = B_FULL // NCORES          # 16 batch per core
TB = 32                        # superblock length
NSB = 8                        # superblocks (Tpad = 256)
TPAD = NSB * TB
F = H * NB                     # 2048 features, branch-major
NM = F // 128                  # 16 feature tiles
KC = 6                         # K chunks (700 inputs + 1 ones-row, padded 768)
USE_F32R = os.environ.get("SNN_F32R", "1") == "1"
REPS = int(os.environ.get("SNN_REPS", "1"))
ABL = set(os.environ.get("SNN_ABLATE", "").split(","))

_PROG = None


def _build_program():
    import concourse.bass as bass
    import concourse.tile as tile
    from concourse import bacc, mybir
    from contextlib import ExitStack

    DT = mybir.dt
    AF = mybir.ActivationFunctionType
    ALU = mybir.AluOpType
    f32 = DT.float32

    nc = bacc.Bacc("TRN2", target_bir_lowering=False, debug=False)

    DTM = DT.float32r if USE_F32R else f32
    xh_d = nc.dram_tensor("xh", [NSB, KC, 128, BC * TB], DTM, kind="ExternalInput")
    w1t_d = nc.dram_tensor("w1t", [KC, 128, F], f32, kind="ExternalInput")
    mkt_d = nc.dram_tensor("mkt", [KC, 128, F], f32, kind="ExternalInput")
    w2t_d = nc.dram_tensor("w2t", [4, 128, OUT], f32, kind="ExternalInput")
    taun_d = nc.dram_tensor("taun", [NB, 1], f32, kind="ExternalInput")
    taum_d = nc.dram_tensor("taum", [1, 1], f32, kind="ExternalInput")
    tauro_d = nc.dram_tensor("tauro", [OUT, 1], f32, kind="ExternalInput")
    b2_d = nc.dram_tensor("b2", [OUT, 1], f32, kind="ExternalInput")
    out_d = nc.dram_tensor("out", [BC, OUT], f32, kind="ExternalOutput")

    with tile.TileContext(nc) as tc, ExitStack() as ctx:
        const = ctx.enter_context(tc.tile_pool(name="const", bufs=1))
        xt_pool = ctx.enter_context(tc.tile_pool(name="xt", bufs=2))
        d_pool = ctx.enter_context(tc.tile_pool(name="dp", bufs=8))
        tmp_pool = ctx.enter_context(tc.tile_pool(name="tp", bufs=3))
        ps_di = ctx.enter_context(tc.tile_pool(name="psdi", bufs=3, space="PSUM"))
        ps_ro = ctx.enter_context(tc.tile_pool(name="psro", bufs=2, space="PSUM"))

        # ---------------- prologue: scalars ----------------
        def load_scalar(src_ap):
            t_ = const.tile([1, 1], f32, tag=f"s{nc.next_id()}", name=f"s{nc.next_id()}")
            nc.sync.dma_start(t_[:], src_ap)
            return t_

        def sig(in_t):
            o = const.tile([1, 1], f32, tag=f"s{nc.next_id()}", name=f"s{nc.next_id()}")
            nc.scalar.activation(o[:], in_t[:], AF.Sigmoid)
            return o

        alpha_s = sig(load_scalar(taum_d.ap()[0:1, :]))
        beta_s = [sig(load_scalar(taun_d.ap()[j:j + 1, :])) for j in range(NB)]
        # 2*(1-alpha)
        am2 = const.tile([1, 1], f32, tag="am2", name="am2")
        nc.vector.tensor_scalar(am2[:], alpha_s[:], -2.0, 2.0, ALU.mult, ALU.add)
        sc_s = []
        for j in range(NB):
            omb = const.tile([1, 1], f32, tag=f"omb{j}", name=f"omb{j}")
            nc.vector.tensor_scalar(omb[:], beta_s[j][:], -1.0, 1.0, ALU.mult, ALU.add)
            sc = const.tile([1, 1], f32, tag=f"sc{j}", name=f"sc{j}")
            nc.vector.tensor_tensor(sc[:], omb[:], am2[:], ALU.mult)
            sc_s.append(sc)

        def bcast(src, tag):
            o = const.tile([128, 1], f32, tag=tag)
            nc.gpsimd.partition_broadcast(o[:], src[:])
            return o

        alpha128 = bcast(alpha_s, "a128")
        beta128 = [bcast(beta_s[j], f"b128_{j}") for j in range(NB)]
        sc128 = [bcast(sc_s[j], f"sc128_{j}") for j in range(NB)]

        # scan reset patterns, one per j-pair: beta_j on its j-slice,
        # 0 at tau=0 of each b-run (kills state at series boundaries)
        pat = []
        for jp in range(2):
            p = const.tile([128, 2, BC, TB], f32, tag=f"pat{jp}", name=f"pat{jp}")
            nc.gpsimd.memset(p[:], 0.0)
            for jj in range(2):
                nc.vector.tensor_scalar(p[:, jj], p[:, jj],
                                        beta128[2 * jp + jj][:], None, ALU.add)
            nc.gpsimd.memset(p[:, :, :, 0], 0.0)
            pat.append(p)

        # ---------------- state ----------------
        carry = const.tile([128, NM, BC], f32, tag="carry", name="carry")
        nc.gpsimd.memset(carry[:], 0.0)
        wv = const.tile([128, 4, BC], f32, tag="wv", name="wv")
        nc.gpsimd.memset(wv[:], 0.0)
        acc = const.tile([OUT, BC], f32, tag="acc", name="acc")
        nc.gpsimd.memset(acc[:], 0.0)
        som = [const.tile([128, 4, BC, TB], f32, tag=f"som{i}", name=f"som{i}") for i in range(2)]
        xn = [const.tile([128, 4, BC, TB], f32, tag=f"xn{i}", name=f"xn{i}") for i in range(2)]

        # ---------------- readout weights C ----------------
        gam = const.tile([OUT, 1], f32, tag="gam", name="gam")
        tro = const.tile([OUT, 1], f32, tag="tro", name="tro")
        nc.sync.dma_start(tro[:], tauro_d.ap())
        nc.scalar.activation(gam[:], tro[:], AF.Sigmoid)
        lng = const.tile([OUT, 1], f32, tag="lng", name="lng")
        nc.scalar.activation(lng[:], gam[:], AF.Ln)
        # column tau of (Y+som'') holds s_{tau-1}; weight it by c_{tau-1},
        # i.e. C'[tau] = (1 - gam^(T+1-tau))/T for tau in [1, T], else 0.
        # xn[0] is idle until the first v-loop; reuse it as C-build scratch
        cscr = xn[0][:].rearrange("p a b t -> p (a b t)")
        iot = cscr[0:OUT, 0:TPAD].bitcast(DT.int32)
        nc.gpsimd.iota(iot, pattern=[[-1, TPAD]], base=T + 1, channel_multiplier=0)
        iotf = cscr[0:OUT, TPAD:2 * TPAD]
        nc.vector.tensor_copy(iotf, iot)
        ctil = cscr[0:OUT, 2 * TPAD:3 * TPAD]
        nc.scalar.activation(ctil, iotf, AF.Exp, scale=lng[:])
        ctau = const.tile([OUT, TPAD], f32, tag="ctau", name="ctau")
        nc.vector.tensor_scalar(ctau[:], ctil, -1.0 / T, 1.0 / T, ALU.mult, ALU.add)
        nc.gpsimd.memset(ctau[:, 0:1], 0.0)
        nc.gpsimd.memset(ctau[:, T + 1:TPAD], 0.0)
        cbig = const.tile([OUT, BC, TPAD], f32, tag="cbig", name="cbig")
        for b in range(BC):
            nc.scalar.copy(cbig[:, b, :], ctau[:])
        scred = const.tile([OUT, 1], f32, tag="scred", name="scred")
        nc.vector.tensor_reduce(scred[:], ctau[:], axis=mybir.AxisListType.X,
                                op=ALU.add)
        b2t = const.tile([OUT, 1], f32, tag="b2t", name="b2t")
        nc.sync.dma_start(b2t[:], b2_d.ap())
        b2term = const.tile([OUT, 1], f32, tag="b2term", name="b2term")
        nc.vector.tensor_tensor(b2term[:], scred[:], b2t[:], ALU.mult)


        # ---------------- weights: Wm'' = (W1*mask)*sc, transposed layout --------
        wt = []
        for k in range(KC):
            w_ = const.tile([128, F], DTM, tag=f"wt{k}", name=f"wt{k}")
            # som/xn are idle until the main loop; reuse as W-prep scratch,
            # alternating W buffers so chunk k+1's DMA overlaps chunk k's mult
            wr_ = som[k % 2][:].rearrange("p a b t -> p (a b t)")[:, 0:F]
            m_ = xn[1][:].rearrange("p a b t -> p (a b t)")[:, 0:F]
            nc.sync.dma_start(wr_, w1t_d.ap()[k])
            nc.sync.dma_start(m_, mkt_d.ap()[k])
            nc.vector.tensor_tensor(wr_, wr_, m_, ALU.mult)
            for j in range(NB):
                nc.scalar.activation(w_[:, j * H:(j + 1) * H], wr_[:, j * H:(j + 1) * H],
                                     AF.Copy, scale=sc128[j][:])
            wt.append(w_)
        w2t = []
        for c in range(4):
            w2c = const.tile([128, OUT], f32, tag=f"w2t{c}", name=f"w2t{c}")
            nc.sync.dma_start(w2c[:], w2t_d.ap()[c])
            w2t.append(w2c)


        KSZ = [128] * 5 + [61]

        def compute_sb(s, defer=False):
            """matmuls + scans + branch sum for superblock s.
            With defer=True, returns closures for the per-(c,j) work so the
            caller can interleave emission with the v-loop chain."""
            xts = []
            for k in range(KC):
                xt = xt_pool.tile([128, BC * TB], DTM, tag=f"xt{k}", name=f"xt{k}")
                nc.sync.dma_start(xt[:], xh_d.ap()[s, k])
                xts.append(xt)
            for c in range(4):
                dt_ = []
                for jp in range(2):
                    pd = ps_di.tile([128, 2, BC * TB], f32, tag="di", name="di")
                    for jj in (() if "mm" in ABL else range(2)):
                        m = (2 * jp + jj) * 4 + c
                        for k in range(KC):
                            nc.tensor.matmul(
                                pd[:, jj],
                                lhsT=wt[k][0:KSZ[k], m * 128:(m + 1) * 128],
                                rhs=xts[k][0:KSZ[k], :],
                                start=(k == 0), stop=(k == KC - 1),
                            )
                    pd4 = pd[:].rearrange("p j (b t) -> p j b t", b=BC)
                    # inject beta-prescaled cross-superblock carries (tau=0)
                    m0 = 2 * jp * 4 + c
                    nc.vector.tensor_tensor(
                        pd4[:, :, :, 0], pd4[:, :, :, 0],
                        carry[:, m0:m0 + 5:4, :], ALU.add)
                    d_ = d_pool.tile([128, 2, BC, TB], f32, tag="d", name="d")
                    nc.vector.tensor_tensor_scan(
                        d_[:].rearrange("p j b t -> p (j b t)"),
                        pat[jp][:].rearrange("p j b t -> p (j b t)"),
                        pd[:].rearrange("p j bt -> p (j bt)"),
                        initial=0.0, op0=ALU.mult, op1=ALU.add)
                    for jj in range(2):
                        m = (2 * jp + jj) * 4 + c
                        nc.scalar.activation(carry[:, m, :], d_[:, jj, :, TB - 1],
                                             AF.Copy, scale=beta128[2 * jp + jj][:])
                    dt_.append(d_)
                t01 = tmp_pool.tile([128, BC, TB], f32, tag="t01", name="t01")
                t23 = tmp_pool.tile([128, BC, TB], f32, tag="t23", name="t23")
                nc.gpsimd.tensor_tensor(t01[:], dt_[0][:, 0], dt_[0][:, 1], ALU.add)
                nc.gpsimd.tensor_tensor(t23[:], dt_[1][:, 0], dt_[1][:, 1], ALU.add)
                nc.gpsimd.tensor_tensor(som[s % 2][:, c], t01[:], t23[:], ALU.add)
            return []

        # two independent per-element chains (c-halves) on DVE and Pool
        VENG = [nc.vector]
        VSL = [(slice(0, 4))]

        def vloop(s, deferred=()):
            deferred = list(deferred)
            wi = 0
            if "vloop" in ABL:
                for w in deferred:
                    w()
                return
            if s == 0:
                for eng, cs in zip(VENG, VSL):
                    eng.tensor_scalar(xn[0][:, cs, :, 0], som[0][:, cs, :, 0],
                                      -1.0, None, ALU.mult)
            for tl in range(TB):
                t = s * TB + tl
                if t > T - 1:
                    break
                for eng, cs in zip(VENG, VSL):
                    eng.scalar_tensor_tensor(
                        wv[:, cs, :], wv[:, cs, :], alpha128[:],
                        xn[s % 2][:, cs, :, tl],
                        op0=ALU.mult, op1=ALU.subtract)
                tn = t + 1
                if tn <= T:
                    s2, tl2 = divmod(tn, TB)
                    for eng, cs in zip(VENG, VSL):
                        eng.scalar_tensor_tensor(
                            xn[s2 % 2][:, cs, :, tl2], wv[:, cs, :], 1.0,
                            som[s2 % 2][:, cs, :, tl2],
                            op0=ALU.is_ge, op1=ALU.subtract)
                if wi < len(deferred):
                    deferred[wi]()
                    wi += 1
            while wi < len(deferred):
                deferred[wi]()
                wi += 1

        def readout(s):
            if "ro" in ABL:
                return
            P = ps_ro.tile([OUT, BC * TB], f32, tag="P", name="P")
            first = True
            for c in range(4):
                for src in (xn[s % 2], som[s % 2]):
                    s2d = src[:].rearrange("p c b t -> p (c b t)")
                    nc.tensor.matmul(
                        P[:],
                        lhsT=w2t[c][:],
                        rhs=s2d[:, c * BC * TB:(c + 1) * BC * TB],
                        start=first, stop=(c == 3 and src is som[s % 2]))
                    first = False
            p3 = P[:].rearrange("p (b t) -> p b t", b=BC)
            nc.vector.tensor_tensor(
                p3, p3, cbig[:, :, s * TB:(s + 1) * TB], ALU.mult)
            res = tmp_pool.tile([OUT, BC], f32, tag="res", name="res")
            nc.vector.tensor_reduce(res[:], p3, axis=mybir.AxisListType.X,
                                    op=ALU.add)
            nc.vector.tensor_tensor(acc[:], acc[:], res[:], ALU.add)

        for _rep in range(REPS):
            compute_sb(0)
            compute_sb(1)
            for s in range(NSB):
                vloop(s)
                readout(s)
                if s + 2 < NSB:
                    compute_sb(s + 2)

        final = const.tile([OUT, BC], f32, tag="final", name="final")
        nc.vector.tensor_scalar(final[:], acc[:], b2term[:], None, ALU.add)
        nc.sync.dma_start(out_d.ap().rearrange("b o -> o b"), final[:])

    nc.compile()
    return nc


def get_program():
    global _PROG
    if _PROG is None:
        _PROG = _build_program()
    return _PROG


def make_in_maps(x, W1, b1, tau_n, tau_m_h, W2, b2, tau_m_ro, mask):
    """Host-side marshalling: reorder/transpose/pad into device layouts."""
    f4 = np.float32
    # feature reorder g=h*NB+j -> f'=j*H+h, then transpose to (IN, F)
    W1r = np.ascontiguousarray(W1.reshape(H, NB, INP).transpose(1, 0, 2)).reshape(F, INP)
    mkr = np.ascontiguousarray(mask.reshape(H, NB, INP).transpose(1, 0, 2)).reshape(F, INP)
    b1r = np.ascontiguousarray(b1.reshape(H, NB).T).reshape(F)
    w1t = np.zeros((KC * 128, F), f4)
    mkt = np.zeros((KC * 128, F), f4)
    w1t[:INP] = W1r.T
    mkt[:INP] = mkr.T
    w1t[INP] = b1r          # bias row (x ones-row at K index 700)
    mkt[INP] = 1.0
    w1t = w1t.reshape(KC, 128, F)
    mkt = mkt.reshape(KC, 128, F)
    w2t = np.ascontiguousarray(W2.T).reshape(4, 128, OUT).astype(f4)
    taun = np.asarray(tau_n, f4).reshape(NB, 1)
    taum = np.asarray(tau_m_h, f4).reshape(1, 1)
    tauro = np.asarray(tau_m_ro, f4).reshape(OUT, 1)
    b2r = np.asarray(b2, f4).reshape(OUT, 1)

    xp = np.zeros((B_FULL, TPAD, KC * 128), f4)
    xp[:, :T, :INP] = x
    xp[:, :, INP] = 1.0     # ones-row for bias
    # (B, TPAD, K) -> per core (NSB, KC, 128, BC, TB)
    xpc = xp.reshape(NCORES, BC, NSB, TB, KC, 128)
    xh = np.ascontiguousarray(xpc.transpose(0, 2, 4, 5, 1, 3)).reshape(
        NCORES, NSB, KC, 128, BC * TB)

    in_maps = []
    for cidx in range(NCORES):
        in_maps.append({
            "xh": xh[cidx], "w1t": w1t, "mkt": mkt, "w2t": w2t,
            "taun": taun, "taum": taum, "tauro": tauro, "b2": b2r,
        })
    return in_maps


def kernel(x, W1, b1, tau_n, tau_m_h, W2, b2, tau_m_ro, mask):
    x = np.asarray(x, np.float32)
    from concourse import bass_utils
    nc = get_program()
    in_maps = make_in_maps(x, W1, b1, tau_n, tau_m_h, W2, b2, tau_m_ro, mask)
    res = bass_utils.run_bass_kernel_spmd(nc, in_maps, core_ids=list(range(NCORES)))
    return np.concatenate([res.results[c]["out"] for c in range(NCORES)], axis=0)



# revision 2
# speedup vs baseline: 42.1848x; 42.1848x over previous
"""DH-SNN network kernel for Trainium2 (8 NeuronCores, batch-parallel).

Math (per core, batch shard Bc=16):
  Wm = (W1*mask) reordered branch-major (f' = j*512+h), scaled by
       sc_j = 2*(1-alpha)*(1-beta_j); bias row b1*sc appended as K-row 700
       with a matching ones-row in x.  (prepared on HOST, single tensor)
  DI'[f,(b,t)] = xT @ Wm''  (PE matmuls, features on partitions)
  d-scan over time per (f,b) series:  d = beta_j*d + DI'   (DVE tensor_tensor_scan,
       reset pattern data0 handles per-b series boundaries; cross-superblock
       carry injected into the tau=0 column of DI')
  som''[h,(b,t)] = sum_j d  (branch sum, scaled s.t. som'' = 2*(1-alpha)*som)
  v-loop (w = 2*v):  w_t = alpha*w_{t-1} - Y_t;   Y_{t+1} = (w_t >= 1) - som''_{t+1}
       (2 scalar_tensor_tensor ops per step; spike s_t = Y_t + som''_t)
  readout: out[b,o] = sum_t c_{t,o} * (s_t @ W2.T)[b,o] + b2[o]*sum_t c_{t,o},
       c_{t,o} = (1 - gam_o^(250-t))/250.  Linear in s => P = W2T@(Y+som'')
       accumulated on PE, then weighted time-reduce on DVE.
All scalar/weight preprocessing (sigmoid decays, W1*mask*sc fold, readout
weight table) is done on the host so the device prologue is DMA-only and the
PE ramps to full occupancy immediately.
"""

import os
import numpy as np

B_FULL, T, INP, H, NB, OUT = 128, 250, 700, 512, 4, 20
NCORES = 8
BC = B_FULL // NCORES          # 16 batch per core
TB = 32                        # superblock length
NSB = 8                        # superblocks (Tpad = 256)
TPAD = NSB * TB
F = H * NB                     # 2048 features, branch-major
NM = F // 128                  # 16 feature tiles
KC = 6                         # K chunks (700 inputs + 1 ones-row, padded 768)
USE_F32R = os.environ.get("SNN_F32R", "1") == "1"
REPS = int(os.environ.get("SNN_REPS", "1"))

_PROG = None


def _build_program(reps=None):
    import concourse.bass as bass
    import concourse.tile as tile
    from concourse import bacc, mybir
    from contextlib import ExitStack

    if reps is None:
        reps = REPS
    DT = mybir.dt
    AF = mybir.ActivationFunctionType
    ALU = mybir.AluOpType
    f32 = DT.float32

    nc = bacc.Bacc("TRN2", target_bir_lowering=False, debug=False)

    DTM = DT.float32r if USE_F32R else f32
    xh_d = nc.dram_tensor("xh", [NSB, KC, 128, BC * TB], DTM, kind="ExternalInput")
    wt_d = nc.dram_tensor("wt", [KC, 128, F], DTM, kind="ExternalInput")
    w2t_d = nc.dram_tensor("w2t", [4, 128, OUT], f32, kind="ExternalInput")
    # host-precomputed small tensors
    pat_d = nc.dram_tensor("pat", [2, 128, 2 * BC * TB], f32, kind="ExternalInput")
    ab_d = nc.dram_tensor("ab", [128, 5], f32, kind="ExternalInput")  # alpha, beta0..3
    cbig_d = nc.dram_tensor("cbig", [OUT, BC * TPAD], f32, kind="ExternalInput")
    b2t_d = nc.dram_tensor("b2t", [OUT, 1], f32, kind="ExternalInput")
    out_d = nc.dram_tensor("out", [BC, OUT], f32, kind="ExternalOutput")

    with tile.TileContext(nc) as tc, ExitStack() as ctx:
        const = ctx.enter_context(tc.tile_pool(name="const", bufs=1))
        xt_pool = ctx.enter_context(tc.tile_pool(name="xt", bufs=2))
        d_pool = ctx.enter_context(tc.tile_pool(name="dp", bufs=8))
        tmp_pool = ctx.enter_context(tc.tile_pool(name="tp", bufs=3))
        ps_di = ctx.enter_context(tc.tile_pool(name="psdi", bufs=3, space="PSUM"))
        ps_ro = ctx.enter_context(tc.tile_pool(name="psro", bufs=2, space="PSUM"))

        # ------------- prologue: pure DMA (first: what gates the first matmul)
        wt = []
        for k in range(KC):
            w_ = const.tile([128, F], DTM, tag=f"wt{k}", name=f"wt{k}")
            nc.sync.dma_start(w_[:], wt_d.ap()[k])
            wt.append(w_)
        pat = []
        for jp in range(2):
            p = const.tile([128, 2, BC, TB], f32, tag=f"pat{jp}", name=f"pat{jp}")
            nc.sync.dma_start(p[:].rearrange("p j b t -> p (j b t)"), pat_d.ap()[jp])
            pat.append(p)
        ab = const.tile([128, 5], f32, tag="ab", name="ab")
        nc.sync.dma_start(ab[:], ab_d.ap())
        alpha128 = ab[:, 0:1]
        beta128 = [ab[:, 1 + j:2 + j] for j in range(NB)]
        w2t = []
        for c in range(4):
            w2c = const.tile([128, OUT], f32, tag=f"w2t{c}", name=f"w2t{c}")
            nc.sync.dma_start(w2c[:], w2t_d.ap()[c])
            w2t.append(w2c)
        cbig = const.tile([OUT, BC, TPAD], f32, tag="cbig", name="cbig")
        nc.sync.dma_start(cbig[:].rearrange("p b t -> p (b t)"), cbig_d.ap())
        b2term = const.tile([OUT, 1], f32, tag="b2term", name="b2term")
        nc.sync.dma_start(b2term[:], b2t_d.ap())

        # ---------------- state ----------------
        carry = const.tile([128, NM, BC], f32, tag="carry", name="carry")
        nc.gpsimd.memset(carry[:], 0.0)
        wv = const.tile([128, 4, BC], f32, tag="wv", name="wv")
        nc.gpsimd.memset(wv[:], 0.0)
        acc = const.tile([OUT, BC], f32, tag="acc", name="acc")
        nc.gpsimd.memset(acc[:], 0.0)
        som = [const.tile([128, 4, BC, TB], f32, tag=f"som{i}", name=f"som{i}") for i in range(2)]
        xn = [const.tile([128, 4, BC, TB], f32, tag=f"xn{i}", name=f"xn{i}") for i in range(2)]

        KSZ = [128] * 5 + [61]

        def compute_sb(s):
            """matmuls + scans + branch sum for superblock s."""
            xts = []
            for k in range(KC):
                xt = xt_pool.tile([128, BC * TB], DTM, tag=f"xt{k}", name=f"xt{k}")
                nc.sync.dma_start(xt[:], xh_d.ap()[s, k])
                xts.append(xt)
            for c in range(4):
                dt_ = []
                for jp in range(2):
                    pd = ps_di.tile([128, 2, BC * TB], f32, tag="di", name="di")
                    for jj in range(2):
                        m = (2 * jp + jj) * 4 + c
                        for k in range(KC):
                            nc.tensor.matmul(
                                pd[:, jj],
                                lhsT=wt[k][0:KSZ[k], m * 128:(m + 1) * 128],
                                rhs=xts[k][0:KSZ[k], :],
                                start=(k == 0), stop=(k == KC - 1),
                            )
                    pd4 = pd[:].rearrange("p j (b t) -> p j b t", b=BC)
                    # inject beta-prescaled cross-superblock carries (tau=0)
                    m0 = 2 * jp * 4 + c
                    nc.vector.tensor_tensor(
                        pd4[:, :, :, 0], pd4[:, :, :, 0],
                        carry[:, m0:m0 + 5:4, :], ALU.add)
                    d_ = d_pool.tile([128, 2, BC, TB], f32, tag="d", name="d")
                    nc.vector.tensor_tensor_scan(
                        d_[:].rearrange("p j b t -> p (j b t)"),
                        pat[jp][:].rearrange("p j b t -> p (j b t)"),
                        pd[:].rearrange("p j bt -> p (j bt)"),
                        initial=0.0, op0=ALU.mult, op1=ALU.add)
                    for jj in range(2):
                        m = (2 * jp + jj) * 4 + c
                        nc.scalar.activation(carry[:, m, :], d_[:, jj, :, TB - 1],
                                             AF.Copy, scale=beta128[2 * jp + jj])
                    dt_.append(d_)
                t01 = tmp_pool.tile([128, BC, TB], f32, tag="t01", name="t01")
                t23 = tmp_pool.tile([128, BC, TB], f32, tag="t23", name="t23")
                nc.gpsimd.tensor_tensor(t01[:], dt_[0][:, 0], dt_[0][:, 1], ALU.add)
                nc.gpsimd.tensor_tensor(t23[:], dt_[1][:, 0], dt_[1][:, 1], ALU.add)
                nc.gpsimd.tensor_tensor(som[s % 2][:, c], t01[:], t23[:], ALU.add)

        def vloop(s):
            if s == 0:
                nc.vector.tensor_scalar(xn[0][:, :, :, 0], som[0][:, :, :, 0],
                                        -1.0, None, ALU.mult)
            for tl in range(TB):
                t = s * TB + tl
                if t > T - 1:
                    break
                nc.vector.scalar_tensor_tensor(
                    wv[:], wv[:], alpha128,
                    xn[s % 2][:, :, :, tl],
                    op0=ALU.mult, op1=ALU.subtract)
                tn = t + 1
                if tn <= T:
                    s2, tl2 = divmod(tn, TB)
                    nc.vector.scalar_tensor_tensor(
                        xn[s2 % 2][:, :, :, tl2], wv[:], 1.0,
                        som[s2 % 2][:, :, :, tl2],
                        op0=ALU.is_ge, op1=ALU.subtract)

        def readout(s):
            P = ps_ro.tile([OUT, BC * TB], f32, tag="P", name="P")
            first = True
            for c in range(4):
                for src in (xn[s % 2], som[s % 2]):
                    s2d = src[:].rearrange("p c b t -> p (c b t)")
                    nc.tensor.matmul(
                        P[:],
                        lhsT=w2t[c][:],
                        rhs=s2d[:, c * BC * TB:(c + 1) * BC * TB],
                        start=first, stop=(c == 3 and src is som[s % 2]))
                    first = False
            p3 = P[:].rearrange("p (b t) -> p b t", b=BC)
            nc.vector.tensor_tensor(
                p3, p3, cbig[:, :, s * TB:(s + 1) * TB], ALU.mult)
            res = tmp_pool.tile([OUT, BC], f32, tag="res", name="res")
            nc.vector.tensor_reduce(res[:], p3, axis=mybir.AxisListType.X,
                                    op=ALU.add)
            nc.vector.tensor_tensor(acc[:], acc[:], res[:], ALU.add)

        for _rep in range(reps):
            compute_sb(0)
            compute_sb(1)
            for s in range(NSB):
                vloop(s)
                readout(s)
                if s + 2 < NSB:
                    compute_sb(s + 2)

        final = const.tile([OUT, BC], f32, tag="final", name="final")
        nc.vector.tensor_scalar(final[:], acc[:], b2term[:], None, ALU.add)
        nc.sync.dma_start(out_d.ap().rearrange("b o -> o b"), final[:])

    nc.compile()
    return nc


def get_program(reps=None):
    global _PROG
    if reps is not None:
        return _build_program(reps)
    if _PROG is None:
        _PROG = _build_program()
    return _PROG


def make_in_maps(x, W1, b1, tau_n, tau_m_h, W2, b2, tau_m_ro, mask):
    """Host-side marshalling: fold scales into weights, reorder/transpose/pad
    into device layouts."""
    f4 = np.float32

    def sigmoid(z):
        return 1.0 / (1.0 + np.exp(-np.asarray(z, np.float64)))

    beta = sigmoid(tau_n).astype(f4)              # (NB,)
    alpha = f4(sigmoid(tau_m_h))                  # scalar
    gam = sigmoid(tau_m_ro).astype(f4)            # (OUT,)
    sc = (2.0 * (1.0 - alpha) * (1.0 - beta)).astype(f4)  # (NB,)

    # feature reorder g=h*NB+j -> f'=j*H+h; fold sc_j; transpose to (IN, F)
    Wm = (np.asarray(W1, f4) * np.asarray(mask, f4))
    W1r = np.ascontiguousarray(Wm.reshape(H, NB, INP).transpose(1, 0, 2))
    W1r *= sc[:, None, None]
    W1r = W1r.reshape(F, INP)
    b1r = np.ascontiguousarray(np.asarray(b1, f4).reshape(H, NB).T).reshape(F)
    b1r = b1r * np.repeat(sc, H)
    wt = np.zeros((KC * 128, F), f4)
    wt[:INP] = W1r.T
    wt[INP] = b1r           # bias row (x ones-row at K index 700)
    wt = wt.reshape(KC, 128, F)
    w2t = np.ascontiguousarray(np.asarray(W2, f4).T).reshape(4, 128, OUT).astype(f4)

    # scan reset patterns: beta_j everywhere on the j-slice, 0 at tau=0
    patv = np.zeros((2, 128, 2, BC, TB), f4)
    for jp in range(2):
        for jj in range(2):
            patv[jp, :, jj, :, 1:] = beta[2 * jp + jj]
    patv = patv.reshape(2, 128, 2 * BC * TB)

    ab = np.zeros((128, 5), f4)
    ab[:, 0] = alpha
    ab[:, 1:5] = beta[None, :]

    # readout weight table: column tau of (Y+som'') holds s_{tau-1}; weight by
    # C[tau] = (1 - gam^(T+1-tau))/T for tau in [1, T], else 0.
    ctau = np.zeros((OUT, TPAD), f4)
    taus = np.arange(1, T + 1)
    ctau[:, 1:T + 1] = (1.0 - gam[:, None] ** (T + 1 - taus)[None, :]) / T
    cbig = np.broadcast_to(ctau[:, None, :], (OUT, BC, TPAD)).reshape(OUT, BC * TPAD)
    cbig = np.ascontiguousarray(cbig)
    b2t = (np.asarray(b2, f4) * ctau.sum(1)).reshape(OUT, 1)

    xp = np.zeros((B_FULL, TPAD, KC * 128), f4)
    xp[:, :T, :INP] = x
    xp[:, :, INP] = 1.0     # ones-row for bias
    # (B, TPAD, K) -> per core (NSB, KC, 128, BC, TB)
    xpc = xp.reshape(NCORES, BC, NSB, TB, KC, 128)
    xh = np.ascontiguousarray(xpc.transpose(0, 2, 4, 5, 1, 3)).reshape(
        NCORES, NSB, KC, 128, BC * TB)

    in_maps = []
    for cidx in range(NCORES):
        in_maps.append({
            "xh": xh[cidx], "wt": wt, "w2t": w2t, "pat": patv,
            "ab": ab, "cbig": cbig, "b2t": b2t,
        })
    return in_maps


def kernel(x, W1, b1, tau_n, tau_m_h, W2, b2, tau_m_ro, mask):
    x = np.asarray(x, np.float32)
    from concourse import bass_utils
    nc = get_program()
    in_maps = make_in_maps(x, W1, b1, tau_n, tau_m_h, W2, b2, tau_m_ro, mask)
    res = bass_utils.run_bass_kernel_spmd(nc, in_maps, core_ids=list(range(NCORES)))
    return np.concatenate([res.results[c]["out"] for c in range(NCORES)], axis=0)


# revision 8
# speedup vs baseline: 48.5116x; 1.1500x over previous
"""DH-SNN network kernel for Trainium2 (8 NeuronCores, batch-parallel).

Math (per core, batch shard Bc=16):
  Wm = (W1*mask) reordered branch-major (f' = j*512+h), scaled by
       sc_j = 2*(1-alpha)*(1-beta_j); bias row b1*sc appended as K-row 700
       with a matching ones-row in x.  (prepared on HOST, single tensor)
  DI'[f,(b,t)] = xT @ Wm''  (PE matmuls, features on partitions)
  d-scan over time per (f,b) series:  d = beta_j*d + DI'   (DVE tensor_tensor_scan,
       reset pattern data0 handles per-b series boundaries; cross-superblock
       carry injected into the tau=0 column of DI')
  som''[h,(b,t)] = sum_j d  (branch sum, scaled s.t. som'' = 2*(1-alpha)*som)
  v-loop (w = 2*v):  w_t = alpha*w_{t-1} - Y_t;   Y_{t+1} = (w_t >= 1) - som''_{t+1}
       (2 scalar_tensor_tensor ops per step; spike s_t = Y_t + som''_t)
  readout: out[b,o] = sum_t c_{t,o} * (s_t @ W2.T)[b,o] + b2[o]*sum_t c_{t,o},
       c_{t,o} = (1 - gam_o^(250-t))/250.  Linear in s => P = W2T@(Y+som'')
       accumulated on PE, then weighted time-reduce on DVE.
All scalar/weight preprocessing (sigmoid decays, W1*mask*sc fold, readout
weight table) is done on the host so the device prologue is DMA-only and the
PE ramps to full occupancy immediately.
"""

import os
import numpy as np

B_FULL, T, INP, H, NB, OUT = 128, 250, 700, 512, 4, 20
NCORES = 8
BC = B_FULL // NCORES          # 16 batch per core
TB = 32                        # superblock length
NSB = 8                        # superblocks (Tpad = 256)
TPAD = NSB * TB
F = H * NB                     # 2048 features, branch-major
NM = F // 128                  # 16 feature tiles
KC = 6                         # K chunks (700 inputs + 1 ones-row, padded 768)
USE_F32R = os.environ.get("SNN_F32R", "1") == "1"
# matmul input dtype: fp16 halves the x/W stream bytes (the per-forward
# bottleneck) at ~6e-3 quantization cost; bf16 would cost ~1.5e-2.
MM_DT = os.environ.get("SNN_MM_DT", "fp16")
if os.environ.get("SNN_BF16", "0") == "1":
    MM_DT = "bf16"
REPS = int(os.environ.get("SNN_REPS", "1"))

_PROG = None


def _build_program(reps=None):
    import concourse.bass as bass
    import concourse.tile as tile
    from concourse import bacc, mybir
    from contextlib import ExitStack

    if reps is None:
        reps = REPS
    DT = mybir.dt
    AF = mybir.ActivationFunctionType
    ALU = mybir.AluOpType
    f32 = DT.float32

    nc = bacc.Bacc("TRN2", target_bir_lowering=False, debug=False)

    DTM = {"bf16": DT.bfloat16, "fp16": DT.float16}.get(
        MM_DT, DT.float32r if USE_F32R else f32)
    xh_d = nc.dram_tensor("xh", [NSB, KC, 128, BC * TB], DTM, kind="ExternalInput")
    wt_d = nc.dram_tensor("wt", [KC, 128, F], DTM, kind="ExternalInput")
    w2t_d = nc.dram_tensor("w2t", [4, 128, OUT], f32, kind="ExternalInput")
    # host-precomputed small tensors
    pat_d = nc.dram_tensor("pat", [2, 128, 2 * BC * TB], f32, kind="ExternalInput")
    ab_d = nc.dram_tensor("ab", [128, 5], f32, kind="ExternalInput")  # alpha, beta0..3
    cbig_d = nc.dram_tensor("cbig", [OUT, BC * TPAD], f32, kind="ExternalInput")
    b2t_d = nc.dram_tensor("b2t", [OUT, 1], f32, kind="ExternalInput")
    out_d = nc.dram_tensor("out", [BC, OUT], f32, kind="ExternalOutput")

    with tile.TileContext(nc) as tc, ExitStack() as ctx:
        const = ctx.enter_context(tc.tile_pool(name="const", bufs=1))
        xt_pool = ctx.enter_context(tc.tile_pool(name="xt", bufs=2))
        d_pool = ctx.enter_context(tc.tile_pool(name="dp", bufs=8))
        tmp_pool = ctx.enter_context(tc.tile_pool(name="tp", bufs=3))
        ps_di = ctx.enter_context(tc.tile_pool(name="psdi", bufs=3, space="PSUM"))
        ps_ro = ctx.enter_context(tc.tile_pool(name="psro", bufs=2, space="PSUM"))

        # ------------- prologue: pure DMA (first: what gates the first matmul)
        wt = []
        for k in range(KC):
            w_ = const.tile([128, F], DTM, tag=f"wt{k}", name=f"wt{k}")
            nc.sync.dma_start(w_[:], wt_d.ap()[k])
            wt.append(w_)
        pat = []
        for jp in range(2):
            p = const.tile([128, 2, BC, TB], f32, tag=f"pat{jp}", name=f"pat{jp}")
            nc.sync.dma_start(p[:].rearrange("p j b t -> p (j b t)"), pat_d.ap()[jp])
            pat.append(p)
        ab = const.tile([128, 5], f32, tag="ab", name="ab")
        nc.sync.dma_start(ab[:], ab_d.ap())
        alpha128 = ab[:, 0:1]
        beta128 = [ab[:, 1 + j:2 + j] for j in range(NB)]
        w2t = []
        for c in range(4):
            w2c = const.tile([128, OUT], f32, tag=f"w2t{c}", name=f"w2t{c}")
            nc.sync.dma_start(w2c[:], w2t_d.ap()[c])
            w2t.append(w2c)
        cbig = const.tile([OUT, BC, TPAD], f32, tag="cbig", name="cbig")
        nc.sync.dma_start(cbig[:].rearrange("p b t -> p (b t)"), cbig_d.ap())
        b2term = const.tile([OUT, 1], f32, tag="b2term", name="b2term")
        nc.sync.dma_start(b2term[:], b2t_d.ap())

        # ---------------- state ----------------
        carry = const.tile([128, NM, BC], f32, tag="carry", name="carry")
        nc.gpsimd.memset(carry[:], 0.0)
        wv = const.tile([128, 4, BC], f32, tag="wv", name="wv")
        nc.gpsimd.memset(wv[:], 0.0)
        acc = const.tile([OUT, BC], f32, tag="acc", name="acc")
        nc.gpsimd.memset(acc[:], 0.0)
        som = [const.tile([128, 4, BC, TB], f32, tag=f"som{i}", name=f"som{i}") for i in range(2)]
        xn = [const.tile([128, 4, BC, TB], f32, tag=f"xn{i}", name=f"xn{i}") for i in range(2)]

        KSZ = [128] * 5 + [61]

        def compute_sb(s):
            """matmuls + scans + branch sum for superblock s."""
            xts = []
            for k in range(KC):
                xt = xt_pool.tile([128, BC * TB], DTM, tag=f"xt{k}", name=f"xt{k}")
                nc.sync.dma_start(xt[:], xh_d.ap()[s, k])
                xts.append(xt)
            for c in range(4):
                dt_ = []
                for jp in range(2):
                    pd = ps_di.tile([128, 2, BC * TB], f32, tag="di", name="di")
                    for jj in range(2):
                        m = (2 * jp + jj) * 4 + c
                        for k in range(KC):
                            nc.tensor.matmul(
                                pd[:, jj],
                                lhsT=wt[k][0:KSZ[k], m * 128:(m + 1) * 128],
                                rhs=xts[k][0:KSZ[k], :],
                                start=(k == 0), stop=(k == KC - 1),
                            )
                    pd4 = pd[:].rearrange("p j (b t) -> p j b t", b=BC)
                    # inject beta-prescaled cross-superblock carries (tau=0)
                    m0 = 2 * jp * 4 + c
                    nc.vector.tensor_tensor(
                        pd4[:, :, :, 0], pd4[:, :, :, 0],
                        carry[:, m0:m0 + 5:4, :], ALU.add)
                    d_ = d_pool.tile([128, 2, BC, TB], f32, tag="d", name="d")
                    nc.vector.tensor_tensor_scan(
                        d_[:].rearrange("p j b t -> p (j b t)"),
                        pat[jp][:].rearrange("p j b t -> p (j b t)"),
                        pd[:].rearrange("p j bt -> p (j bt)"),
                        initial=0.0, op0=ALU.mult, op1=ALU.add)
                    for jj in range(2):
                        m = (2 * jp + jj) * 4 + c
                        nc.scalar.activation(carry[:, m, :], d_[:, jj, :, TB - 1],
                                             AF.Copy, scale=beta128[2 * jp + jj])
                    dt_.append(d_)
                t01 = tmp_pool.tile([128, BC, TB], f32, tag="t01", name="t01")
                t23 = tmp_pool.tile([128, BC, TB], f32, tag="t23", name="t23")
                nc.gpsimd.tensor_tensor(t01[:], dt_[0][:, 0], dt_[0][:, 1], ALU.add)
                nc.gpsimd.tensor_tensor(t23[:], dt_[1][:, 0], dt_[1][:, 1], ALU.add)
                nc.gpsimd.tensor_tensor(som[s % 2][:, c], t01[:], t23[:], ALU.add)

        def vloop(s):
            if s == 0:
                nc.vector.tensor_scalar(xn[0][:, :, :, 0], som[0][:, :, :, 0],
                                        -1.0, None, ALU.mult)
            for tl in range(TB):
                t = s * TB + tl
                if t > T - 1:
                    break
                nc.vector.scalar_tensor_tensor(
                    wv[:], wv[:], alpha128,
                    xn[s % 2][:, :, :, tl],
                    op0=ALU.mult, op1=ALU.subtract)
                tn = t + 1
                if tn <= T:
                    s2, tl2 = divmod(tn, TB)
                    nc.vector.scalar_tensor_tensor(
                        xn[s2 % 2][:, :, :, tl2], wv[:], 1.0,
                        som[s2 % 2][:, :, :, tl2],
                        op0=ALU.is_ge, op1=ALU.subtract)

        def readout(s):
            P = ps_ro.tile([OUT, BC * TB], f32, tag="P", name="P")
            first = True
            for c in range(4):
                for src in (xn[s % 2], som[s % 2]):
                    s2d = src[:].rearrange("p c b t -> p (c b t)")
                    nc.tensor.matmul(
                        P[:],
                        lhsT=w2t[c][:],
                        rhs=s2d[:, c * BC * TB:(c + 1) * BC * TB],
                        start=first, stop=(c == 3 and src is som[s % 2]))
                    first = False
            p3 = P[:].rearrange("p (b t) -> p b t", b=BC)
            nc.vector.tensor_tensor(
                p3, p3, cbig[:, :, s * TB:(s + 1) * TB], ALU.mult)
            res = tmp_pool.tile([OUT, BC], f32, tag="res", name="res")
            nc.vector.tensor_reduce(res[:], p3, axis=mybir.AxisListType.X,
                                    op=ALU.add)
            nc.vector.tensor_tensor(acc[:], acc[:], res[:], ALU.add)

        for _rep in range(reps):
            compute_sb(0)
            compute_sb(1)
            for s in range(NSB):
                vloop(s)
                readout(s)
                if s + 2 < NSB:
                    compute_sb(s + 2)

        final = const.tile([OUT, BC], f32, tag="final", name="final")
        nc.vector.tensor_scalar(final[:], acc[:], b2term[:], None, ALU.add)
        nc.sync.dma_start(out_d.ap().rearrange("b o -> o b"), final[:])

    nc.compile()
    return nc


def get_program(reps=None):
    global _PROG
    if reps is not None:
        return _build_program(reps)
    if _PROG is None:
        _PROG = _build_program()
    return _PROG


def make_in_maps(x, W1, b1, tau_n, tau_m_h, W2, b2, tau_m_ro, mask):
    """Host-side marshalling: fold scales into weights, reorder/transpose/pad
    into device layouts."""
    f4 = np.float32

    def sigmoid(z):
        return 1.0 / (1.0 + np.exp(-np.asarray(z, np.float64)))

    beta = sigmoid(tau_n).astype(f4)              # (NB,)
    alpha = f4(sigmoid(tau_m_h))                  # scalar
    gam = sigmoid(tau_m_ro).astype(f4)            # (OUT,)
    sc = (2.0 * (1.0 - alpha) * (1.0 - beta)).astype(f4)  # (NB,)

    # feature reorder g=h*NB+j -> f'=j*H+h; fold sc_j; transpose to (IN, F)
    Wm = (np.asarray(W1, f4) * np.asarray(mask, f4))
    W1r = np.ascontiguousarray(Wm.reshape(H, NB, INP).transpose(1, 0, 2))
    W1r *= sc[:, None, None]
    W1r = W1r.reshape(F, INP)
    b1r = np.ascontiguousarray(np.asarray(b1, f4).reshape(H, NB).T).reshape(F)
    b1r = b1r * np.repeat(sc, H)
    wt = np.zeros((KC * 128, F), f4)
    wt[:INP] = W1r.T
    wt[INP] = b1r           # bias row (x ones-row at K index 700)
    wt = wt.reshape(KC, 128, F)
    w2t = np.ascontiguousarray(np.asarray(W2, f4).T).reshape(4, 128, OUT).astype(f4)

    # scan reset patterns: beta_j everywhere on the j-slice, 0 at tau=0
    patv = np.zeros((2, 128, 2, BC, TB), f4)
    for jp in range(2):
        for jj in range(2):
            patv[jp, :, jj, :, 1:] = beta[2 * jp + jj]
    patv = patv.reshape(2, 128, 2 * BC * TB)

    ab = np.zeros((128, 5), f4)
    ab[:, 0] = alpha
    ab[:, 1:5] = beta[None, :]

    # readout weight table: column tau of (Y+som'') holds s_{tau-1}; weight by
    # C[tau] = (1 - gam^(T+1-tau))/T for tau in [1, T], else 0.
    ctau = np.zeros((OUT, TPAD), f4)
    taus = np.arange(1, T + 1)
    ctau[:, 1:T + 1] = (1.0 - gam[:, None] ** (T + 1 - taus)[None, :]) / T
    cbig = np.broadcast_to(ctau[:, None, :], (OUT, BC, TPAD)).reshape(OUT, BC * TPAD)
    cbig = np.ascontiguousarray(cbig)
    b2t = (np.asarray(b2, f4) * ctau.sum(1)).reshape(OUT, 1)

    xp = np.zeros((B_FULL, TPAD, KC * 128), f4)
    xp[:, :T, :INP] = x
    xp[:, :, INP] = 1.0     # ones-row for bias
    # (B, TPAD, K) -> per core (NSB, KC, 128, BC, TB)
    xpc = xp.reshape(NCORES, BC, NSB, TB, KC, 128)
    xh = np.ascontiguousarray(xpc.transpose(0, 2, 4, 5, 1, 3)).reshape(
        NCORES, NSB, KC, 128, BC * TB)

    if MM_DT == "bf16":
        import ml_dtypes
        wt = wt.astype(ml_dtypes.bfloat16)
        xh = xh.astype(ml_dtypes.bfloat16)
    elif MM_DT == "fp16":
        wt = wt.astype(np.float16)
        xh = xh.astype(np.float16)

    in_maps = []
    for cidx in range(NCORES):
        in_maps.append({
            "xh": xh[cidx], "wt": wt, "w2t": w2t, "pat": patv,
            "ab": ab, "cbig": cbig, "b2t": b2t,
        })
    return in_maps


def kernel(x, W1, b1, tau_n, tau_m_h, W2, b2, tau_m_ro, mask):
    x = np.asarray(x, np.float32)
    from concourse import bass_utils
    nc = get_program()
    in_maps = make_in_maps(x, W1, b1, tau_n, tau_m_h, W2, b2, tau_m_ro, mask)
    res = bass_utils.run_bass_kernel_spmd(nc, in_maps, core_ids=list(range(NCORES)))
    return np.concatenate([res.results[c]["out"] for c in range(NCORES)], axis=0)


# revision 9
# speedup vs baseline: 51.4041x; 1.0596x over previous
"""DH-SNN network kernel for Trainium2 (8 NeuronCores, batch-parallel).

Math (per core, batch shard Bc=16):
  Wm = (W1*mask) reordered branch-major (f' = j*512+h), scaled by
       sc_j = 2*(1-alpha)*(1-beta_j); bias row b1*sc appended as K-row 700
       with a matching ones-row in x.  (prepared on HOST, single tensor)
  DI'[f,(b,t)] = xT @ Wm''  (PE matmuls, features on partitions)
  d-scan over time per (f,b) series:  d = beta_j*d + DI'   (DVE tensor_tensor_scan,
       reset pattern data0 handles per-b series boundaries; cross-superblock
       carry injected into the tau=0 column of DI')
  som''[h,(b,t)] = sum_j d  (branch sum, scaled s.t. som'' = 2*(1-alpha)*som)
  v-loop (w = 2*v):  w_t = alpha*w_{t-1} - Y_t;   Y_{t+1} = (w_t >= 1) - som''_{t+1}
       (2 scalar_tensor_tensor ops per step; spike s_t = Y_t + som''_t)
  readout: out[b,o] = sum_t c_{t,o} * (s_t @ W2.T)[b,o] + b2[o]*sum_t c_{t,o},
       c_{t,o} = (1 - gam_o^(250-t))/250.  Linear in s => P = W2T@(Y+som'')
       accumulated on PE, then weighted time-reduce on DVE.
All scalar/weight preprocessing (sigmoid decays, W1*mask*sc fold, readout
weight table) is done on the host so the device prologue is DMA-only and the
PE ramps to full occupancy immediately.  Matmul operands (x, Wm) are fp16:
halves the HBM stream per forward at ~6e-3 quantization cost (bf16 would
cost ~1.5e-2 against the 2e-2 gate); scans/v-loop/readout stay fp32.
"""

import os
import numpy as np

B_FULL, T, INP, H, NB, OUT = 128, 250, 700, 512, 4, 20
NCORES = 8
BC = B_FULL // NCORES          # 16 batch per core
TB = 32                        # superblock length
NSB = 8                        # superblocks (Tpad = 256)
TPAD = NSB * TB
F = H * NB                     # 2048 features, branch-major
NM = F // 128                  # 16 feature tiles
KC = 6                         # K chunks (700 inputs + 1 ones-row, padded 768)
USE_F32R = os.environ.get("SNN_F32R", "1") == "1"
# matmul input dtype: fp16 halves the x/W stream bytes (the per-forward
# bottleneck) at ~6e-3 quantization cost; bf16 would cost ~1.5e-2.
MM_DT = os.environ.get("SNN_MM_DT", "fp16")
if os.environ.get("SNN_BF16", "0") == "1":
    MM_DT = "bf16"
REPS = int(os.environ.get("SNN_REPS", "1"))

_PROG = None


def _build_program(reps=None):
    import concourse.bass as bass
    import concourse.tile as tile
    from concourse import bacc, mybir
    from contextlib import ExitStack

    if reps is None:
        reps = REPS
    DT = mybir.dt
    AF = mybir.ActivationFunctionType
    ALU = mybir.AluOpType
    f32 = DT.float32

    nc = bacc.Bacc("TRN2", target_bir_lowering=False, debug=False)

    DTM = {"bf16": DT.bfloat16, "fp16": DT.float16}.get(
        MM_DT, DT.float32r if USE_F32R else f32)
    xh_d = nc.dram_tensor("xh", [NSB, KC, 128, BC * TB], DTM, kind="ExternalInput")
    wt_d = nc.dram_tensor("wt", [KC, 128, F], DTM, kind="ExternalInput")
    w2t_d = nc.dram_tensor("w2t", [4, 128, OUT], f32, kind="ExternalInput")
    # host-precomputed small tensors
    pat_d = nc.dram_tensor("pat", [2, 128, 2 * BC * TB], f32, kind="ExternalInput")
    ab_d = nc.dram_tensor("ab", [128, 5], f32, kind="ExternalInput")  # alpha, beta0..3
    cbig_d = nc.dram_tensor("cbig", [OUT, BC * TPAD], f32, kind="ExternalInput")
    b2t_d = nc.dram_tensor("b2t", [OUT, 1], f32, kind="ExternalInput")
    out_d = nc.dram_tensor("out", [BC, OUT], f32, kind="ExternalOutput")

    with tile.TileContext(nc) as tc, ExitStack() as ctx:
        const = ctx.enter_context(tc.tile_pool(name="const", bufs=1))
        xt_pool = ctx.enter_context(tc.tile_pool(name="xt", bufs=2))
        d_pool = ctx.enter_context(tc.tile_pool(name="dp", bufs=8))
        tmp_pool = ctx.enter_context(tc.tile_pool(name="tp", bufs=3))
        ps_di = ctx.enter_context(tc.tile_pool(name="psdi", bufs=3, space="PSUM"))
        ps_ro = ctx.enter_context(tc.tile_pool(name="psro", bufs=2, space="PSUM"))

        # ------------- prologue: pure DMA (first: what gates the first matmul)
        wt = []
        for k in range(KC):
            w_ = const.tile([128, F], DTM, tag=f"wt{k}", name=f"wt{k}")
            nc.sync.dma_start(w_[:], wt_d.ap()[k])
            wt.append(w_)
        pat = []
        for jp in range(2):
            p = const.tile([128, 2, BC, TB], f32, tag=f"pat{jp}", name=f"pat{jp}")
            nc.sync.dma_start(p[:].rearrange("p j b t -> p (j b t)"), pat_d.ap()[jp])
            pat.append(p)
        ab = const.tile([128, 5], f32, tag="ab", name="ab")
        nc.sync.dma_start(ab[:], ab_d.ap())
        alpha128 = ab[:, 0:1]
        beta128 = [ab[:, 1 + j:2 + j] for j in range(NB)]
        w2t = []
        for c in range(4):
            w2c = const.tile([128, OUT], f32, tag=f"w2t{c}", name=f"w2t{c}")
            nc.sync.dma_start(w2c[:], w2t_d.ap()[c])
            w2t.append(w2c)
        cbig = const.tile([OUT, BC, TPAD], f32, tag="cbig", name="cbig")
        nc.sync.dma_start(cbig[:].rearrange("p b t -> p (b t)"), cbig_d.ap())
        b2term = const.tile([OUT, 1], f32, tag="b2term", name="b2term")
        nc.sync.dma_start(b2term[:], b2t_d.ap())

        # ---------------- state ----------------
        carry = const.tile([128, NM, BC], f32, tag="carry", name="carry")
        nc.gpsimd.memset(carry[:], 0.0)
        wv = const.tile([128, 4, BC], f32, tag="wv", name="wv")
        nc.gpsimd.memset(wv[:], 0.0)
        acc = const.tile([OUT, BC], f32, tag="acc", name="acc")
        nc.gpsimd.memset(acc[:], 0.0)
        som = [const.tile([128, 4, BC, TB], f32, tag=f"som{i}", name=f"som{i}") for i in range(2)]
        xn = [const.tile([128, 4, BC, TB], f32, tag=f"xn{i}", name=f"xn{i}") for i in range(2)]

        KSZ = [128] * 5 + [61]

        def compute_sb(s):
            """matmuls + scans + branch sum for superblock s."""
            xts = []
            for k in range(KC):
                xt = xt_pool.tile([128, BC * TB], DTM, tag=f"xt{k}", name=f"xt{k}")
                nc.sync.dma_start(xt[:], xh_d.ap()[s, k])
                xts.append(xt)
            for c in range(4):
                dt_ = []
                for jp in range(2):
                    pd = ps_di.tile([128, 2, BC * TB], f32, tag="di", name="di")
                    for jj in range(2):
                        m = (2 * jp + jj) * 4 + c
                        for k in range(KC):
                            nc.tensor.matmul(
                                pd[:, jj],
                                lhsT=wt[k][0:KSZ[k], m * 128:(m + 1) * 128],
                                rhs=xts[k][0:KSZ[k], :],
                                start=(k == 0), stop=(k == KC - 1),
                            )
                    pd4 = pd[:].rearrange("p j (b t) -> p j b t", b=BC)
                    # inject beta-prescaled cross-superblock carries (tau=0)
                    m0 = 2 * jp * 4 + c
                    nc.vector.tensor_tensor(
                        pd4[:, :, :, 0], pd4[:, :, :, 0],
                        carry[:, m0:m0 + 5:4, :], ALU.add)
                    d_ = d_pool.tile([128, 2, BC, TB], f32, tag="d", name="d")
                    nc.vector.tensor_tensor_scan(
                        d_[:].rearrange("p j b t -> p (j b t)"),
                        pat[jp][:].rearrange("p j b t -> p (j b t)"),
                        pd[:].rearrange("p j bt -> p (j bt)"),
                        initial=0.0, op0=ALU.mult, op1=ALU.add)
                    for jj in range(2):
                        m = (2 * jp + jj) * 4 + c
                        nc.scalar.activation(carry[:, m, :], d_[:, jj, :, TB - 1],
                                             AF.Copy, scale=beta128[2 * jp + jj])
                    dt_.append(d_)
                t01 = tmp_pool.tile([128, BC, TB], f32, tag="t01", name="t01")
                t23 = tmp_pool.tile([128, BC, TB], f32, tag="t23", name="t23")
                nc.gpsimd.tensor_tensor(t01[:], dt_[0][:, 0], dt_[0][:, 1], ALU.add)
                nc.gpsimd.tensor_tensor(t23[:], dt_[1][:, 0], dt_[1][:, 1], ALU.add)
                nc.gpsimd.tensor_tensor(som[s % 2][:, c], t01[:], t23[:], ALU.add)

        def vloop(s):
            if s == 0:
                nc.vector.tensor_scalar(xn[0][:, :, :, 0], som[0][:, :, :, 0],
                                        -1.0, None, ALU.mult)
            for tl in range(TB):
                t = s * TB + tl
                if t > T - 1:
                    break
                nc.vector.scalar_tensor_tensor(
                    wv[:], wv[:], alpha128,
                    xn[s % 2][:, :, :, tl],
                    op0=ALU.mult, op1=ALU.subtract)
                tn = t + 1
                if tn <= T:
                    s2, tl2 = divmod(tn, TB)
                    nc.vector.scalar_tensor_tensor(
                        xn[s2 % 2][:, :, :, tl2], wv[:], 1.0,
                        som[s2 % 2][:, :, :, tl2],
                        op0=ALU.is_ge, op1=ALU.subtract)

        def readout(s):
            P = ps_ro.tile([OUT, BC * TB], f32, tag="P", name="P")
            first = True
            for c in range(4):
                for src in (xn[s % 2], som[s % 2]):
                    s2d = src[:].rearrange("p c b t -> p (c b t)")
                    nc.tensor.matmul(
                        P[:],
                        lhsT=w2t[c][:],
                        rhs=s2d[:, c * BC * TB:(c + 1) * BC * TB],
                        start=first, stop=(c == 3 and src is som[s % 2]))
                    first = False
            p3 = P[:].rearrange("p (b t) -> p b t", b=BC)
            nc.vector.tensor_tensor(
                p3, p3, cbig[:, :, s * TB:(s + 1) * TB], ALU.mult)
            res = tmp_pool.tile([OUT, BC], f32, tag="res", name="res")
            nc.vector.tensor_reduce(res[:], p3, axis=mybir.AxisListType.X,
                                    op=ALU.add)
            nc.vector.tensor_tensor(acc[:], acc[:], res[:], ALU.add)

        for _rep in range(reps):
            compute_sb(0)
            compute_sb(1)
            for s in range(NSB):
                vloop(s)
                readout(s)
                if s + 2 < NSB:
                    compute_sb(s + 2)

        final = const.tile([OUT, BC], f32, tag="final", name="final")
        nc.vector.tensor_scalar(final[:], acc[:], b2term[:], None, ALU.add)
        nc.sync.dma_start(out_d.ap().rearrange("b o -> o b"), final[:])

    nc.compile()
    return nc


def get_program(reps=None):
    global _PROG
    if reps is not None:
        return _build_program(reps)
    if _PROG is None:
        _PROG = _build_program()
    return _PROG


def make_in_maps(x, W1, b1, tau_n, tau_m_h, W2, b2, tau_m_ro, mask):
    """Host-side marshalling: fold scales into weights, reorder/transpose/pad
    into device layouts."""
    f4 = np.float32

    def sigmoid(z):
        return 1.0 / (1.0 + np.exp(-np.asarray(z, np.float64)))

    beta = sigmoid(tau_n).astype(f4)              # (NB,)
    alpha = f4(sigmoid(tau_m_h))                  # scalar
    gam = sigmoid(tau_m_ro).astype(f4)            # (OUT,)
    sc = (2.0 * (1.0 - alpha) * (1.0 - beta)).astype(f4)  # (NB,)

    # feature reorder g=h*NB+j -> f'=j*H+h; fold sc_j; transpose to (IN, F)
    Wm = (np.asarray(W1, f4) * np.asarray(mask, f4))
    W1r = np.ascontiguousarray(Wm.reshape(H, NB, INP).transpose(1, 0, 2))
    W1r *= sc[:, None, None]
    W1r = W1r.reshape(F, INP)
    b1r = np.ascontiguousarray(np.asarray(b1, f4).reshape(H, NB).T).reshape(F)
    b1r = b1r * np.repeat(sc, H)
    wt = np.zeros((KC * 128, F), f4)
    wt[:INP] = W1r.T
    wt[INP] = b1r           # bias row (x ones-row at K index 700)
    wt = wt.reshape(KC, 128, F)
    w2t = np.ascontiguousarray(np.asarray(W2, f4).T).reshape(4, 128, OUT).astype(f4)

    # scan reset patterns: beta_j everywhere on the j-slice, 0 at tau=0
    patv = np.zeros((2, 128, 2, BC, TB), f4)
    for jp in range(2):
        for jj in range(2):
            patv[jp, :, jj, :, 1:] = beta[2 * jp + jj]
    patv = patv.reshape(2, 128, 2 * BC * TB)

    ab = np.zeros((128, 5), f4)
    ab[:, 0] = alpha
    ab[:, 1:5] = beta[None, :]

    # readout weight table: column tau of (Y+som'') holds s_{tau-1}; weight by
    # C[tau] = (1 - gam^(T+1-tau))/T for tau in [1, T], else 0.
    ctau = np.zeros((OUT, TPAD), f4)
    taus = np.arange(1, T + 1)
    ctau[:, 1:T + 1] = (1.0 - gam[:, None] ** (T + 1 - taus)[None, :]) / T
    cbig = np.broadcast_to(ctau[:, None, :], (OUT, BC, TPAD)).reshape(OUT, BC * TPAD)
    cbig = np.ascontiguousarray(cbig)
    b2t = (np.asarray(b2, f4) * ctau.sum(1)).reshape(OUT, 1)

    xp = np.zeros((B_FULL, TPAD, KC * 128), f4)
    xp[:, :T, :INP] = x
    xp[:, :, INP] = 1.0     # ones-row for bias
    # (B, TPAD, K) -> per core (NSB, KC, 128, BC, TB)
    xpc = xp.reshape(NCORES, BC, NSB, TB, KC, 128)
    xh = np.ascontiguousarray(xpc.transpose(0, 2, 4, 5, 1, 3)).reshape(
        NCORES, NSB, KC, 128, BC * TB)

    if MM_DT == "bf16":
        import ml_dtypes
        wt = wt.astype(ml_dtypes.bfloat16)
        xh = xh.astype(ml_dtypes.bfloat16)
    elif MM_DT == "fp16":
        wt = wt.astype(np.float16)
        xh = xh.astype(np.float16)

    in_maps = []
    for cidx in range(NCORES):
        in_maps.append({
            "xh": xh[cidx], "wt": wt, "w2t": w2t, "pat": patv,
            "ab": ab, "cbig": cbig, "b2t": b2t,
        })
    return in_maps


def kernel(x, W1, b1, tau_n, tau_m_h, W2, b2, tau_m_ro, mask):
    x = np.asarray(x, np.float32)
    from concourse import bass_utils
    nc = get_program()
    in_maps = make_in_maps(x, W1, b1, tau_n, tau_m_h, W2, b2, tau_m_ro, mask)
    res = bass_utils.run_bass_kernel_spmd(nc, in_maps, core_ids=list(range(NCORES)))
    return np.concatenate([res.results[c]["out"] for c in range(NCORES)], axis=0)
